# revision 27
# baseline (speedup 1.0000x reference)
"""Luong local-p attention (scaled-dot, gaussian window) on 8 trn2 cores.

Strategy (data-parallel over batch, 2 examples/core):
  - Host: transpose source_hidden_states to [H, S] per example. Ships an
    fp8-e4m3 copy (streamed once for scores/softmax denominator — the
    denominator sums 4096 exp terms, so the ~3% per-score fp8 noise
    averages out to ~4e-3 on Z), a bf16 copy used only for the 256-wide
    window re-read, and pre-replicated target stationaries (fp8 for the
    score matmuls, bf16 for the window matmuls) so no on-device broadcast
    chain gates the first matmul.
  - Device per example:
      p = S*sigmoid(v_p . tanh(W_p^T t + b_p) + b_v)  (f32r PE matmul + ACT)
      scores[s] = (src[s,:] . t)/sqrt(H)               (fp8 PE, psum-acc)
      softmax denominator Z over full S with a CONSTANT shift of -8 instead
      of the max (scores are ~N(0,1); fp32 range makes a computed max
      unnecessary, and the constant shift cancels exactly in the ratio)
      window [s0, s0+256), s0 = clamp(floor(p)-128, 0, S-256) covers every
      position whose gaussian factor exceeds ~3.3e-4 (4 sigma); window
      scores are recomputed in bf16 from the re-fetched bf16 window columns.
      Context accumulates unnormalized with Z appended as a 9th column; the
      1/Z division happens host-side during unsharding.
  - Queues: the fp8 stream and W_p ride the SP (sync) queue back-to-back —
    nothing s0-gated is emitted ahead of them, so the stream never stalls
    on the p-chain. Window re-reads, the gauss pipeline, and outputs ride
    the ACT (scalar) queue after example 0's denominator exps. Example 1's
    last h-chunk streams in eight 512-column pieces so each score bank's
    exp fires as its piece lands instead of serializing after the stream.
  - Resources: score psum uses 4 banks (two s-blocks per bank, 64-row
    replication via PE column tiling), the p-computation owns 2 banks and
    its 16 f32r matmuls interleave into example 0's score groups, the
    window recompute owns the last 2.
"""

import numpy as np

N_CORES = 8
B, S, H = 16, 4096, 1024
BEX = B // N_CORES  # examples per core
NH = H // 128  # h-chunks of 128 partitions
NSB = S // 512  # s-blocks of 512
NBK = NSB // 2  # psum banks for scores (2 blocks per bank)
WIN = 256
SCALE = 1.0 / 32.0  # 1/sqrt(H)
GEXP = -1.0 / 2048.0  # -1/(2*sigma^2), sigma = WINDOW/2 = 32
EBIAS = -8.0  # constant softmax shift
S0MAX = float(S - WIN)

_CACHE = {}


def _build():
    import concourse.bacc as bacc
    import concourse.bass as bass
    import concourse.mybir as mybir
    import concourse.tile as tile

    f32 = mybir.dt.float32
    f32r = mybir.dt.float32r
    bf16 = mybir.dt.bfloat16
    f8 = mybir.dt.float8e4
    i32 = mybir.dt.int32
    AF = mybir.ActivationFunctionType
    OP = mybir.AluOpType
    AX = mybir.AxisListType
    ET = mybir.EngineType
    ds = bass.ds

    nc = bacc.Bacc("TRN2", target_bir_lowering=False, debug=False, num_devices=N_CORES)
    srcT8 = nc.dram_tensor("srcT8", [BEX, H, S], f8, kind="ExternalInput").ap()
    srcTb = nc.dram_tensor("srcTb", [BEX, H, S], bf16, kind="ExternalInput").ap()
    trep8 = nc.dram_tensor("trep8", [128, BEX, NH, 128], f8, kind="ExternalInput").ap()
    trep16 = nc.dram_tensor("trep16", [128, BEX, NH, 128], bf16, kind="ExternalInput").ap()
    tgtT = nc.dram_tensor("tgtT", [128, NH, BEX], f32r, kind="ExternalInput").ap()
    wp = nc.dram_tensor("wp", [H, H], f32r, kind="ExternalInput").ap()
    pvb = nc.dram_tensor("pvb", [BEX, 2 * H + 2], f32, kind="ExternalInput").ap()
    out = nc.dram_tensor("out", [BEX, 128, NH + 1], f32, kind="ExternalOutput").ap()
    scr_sp = nc.dram_tensor("scr_sp", [BEX, 1], f32).ap()

    with tile.TileContext(nc) as tc:
        with (
            tc.tile_pool(name="cpool", bufs=1) as cpool,
            tc.tile_pool(name="spool", bufs=8) as spool,
            tc.tile_pool(name="winpool", bufs=10) as winpool,
            tc.tile_pool(name="mpool", bufs=2) as mpool,
            tc.tile_pool(name="psB", bufs=1, space="PSUM") as psB,
        ):
            # ---------------- setup ------------------------------------------
            # first stream chunk dispatches ahead of everything else
            big00 = spool.tile([128, S], f8, tag="stream", name="big_0_0")
            nc.sync.dma_start(big00[:], srcT8[0, 0:128, :])

            tr8 = cpool.tile([128, BEX, NH, 128], f8, tag="tr8")
            nc.sync.dma_start(tr8[:], trep8[:])
            t_rep8 = [[tr8[:, e, c, :] for c in range(NH)] for e in range(BEX)]

            # setup that isn't needed for the first matmuls rides ACT
            pvb_sb = cpool.tile([BEX, 2 * H + 2], f32, tag="pvb_sb")
            nc.scalar.dma_start(pvb_sb[:], pvb[:])
            bp_sb = pvb_sb[:, 0:H]
            v_b = pvb_sb[:, H : 2 * H]
            nbv_sb = pvb_sb[:, 2 * H + 1 : 2 * H + 2]
            tr16 = cpool.tile([128, BEX, NH, 128], bf16, tag="tr16")
            nc.scalar.dma_start(tr16[:], trep16[:])
            t_rep16 = [[tr16[:, e, c, :] for c in range(NH)] for e in range(BEX)]
            tTall = cpool.tile([128, NH, BEX], f32r, tag="tTall")
            nc.scalar.dma_start(tTall[:], tgtT[:])
            tT = [tTall[:, c, :] for c in range(NH)]

            wtall = cpool.tile([128, NH, H], f32r, tag="wtall")
            wts = [wtall[:, c, :] for c in range(NH)]

            zeros = cpool.tile([128, 128], f32, tag="zeros")
            nc.vector.memset(zeros[:], 0.0)
            ebias = cpool.tile([128, 1], f32, tag="ebias")
            nc.vector.memset(ebias[:], EBIAS)

            iota_i = cpool.tile([128, WIN], i32, tag="iota_i")
            nc.gpsimd.iota(iota_i[:], pattern=[[1, WIN]], base=0, channel_multiplier=0)
            iota_f = cpool.tile([128, WIN], f32, tag="iota_f")
            nc.vector.tensor_copy(iota_f[:], iota_i[:])

            def emit_score_mms(e, ps, c, big, only_block=None):
                blocks = range(NSB) if only_block is None else [only_block]
                for k in blocks:
                    j, half = divmod(k, 2)
                    pslice = ps[j][64 * half : 64 * (half + 1), :]
                    nc.tensor.matmul(
                        pslice,
                        t_rep8[e][c][:, 0:64],
                        big[:, k * 512 : (k + 1) * 512],
                        start=(c == 0),
                        stop=(c == NH - 1),
                        tile_position=(0, 64 * half),
                        skip_group_check=True,
                    )

            # ---------------- ex0 scores + W_p interleaved on SP queue -------
            ps_hp0 = psB.tile([BEX, 512], f32, tag="hp0", name="hp0")
            ps_hp1 = psB.tile([BEX, 512], f32, tag="hp1", name="hp1")
            ps0 = [
                psB.tile([128, 512], f32, tag=f"scA{j}", name=f"scA{j}_0")
                for j in range(NBK)
            ]
            def hp_mms(c):
                nc.tensor.matmul(
                    ps_hp0[:], tT[c][:], wts[c][:, 0:512], start=(c == 0), stop=(c == NH - 1)
                )
                nc.tensor.matmul(
                    ps_hp1[:], tT[c][:], wts[c][:, 512:1024], start=(c == 0), stop=(c == NH - 1)
                )

            # W_p rides the SP queue right after the first stream chunk, so p
            # is known ~1/3 of the way through the stream; the rest of ex0's
            # stream follows.
            emit_score_mms(0, ps0, 0, big00)
            for c in range(NH):
                nc.sync.dma_start(wtall[:, c, :], wp[c * 128 : (c + 1) * 128, :])
                hp_mms(c)
            for c in range(1, NH - 1):
                big = spool.tile([128, S], f8, tag="stream", name=f"big_0_{c}")
                nc.sync.dma_start(big[:], srcT8[0, c * 128 : (c + 1) * 128, :])
                emit_score_mms(0, ps0, c, big)
            cl = NH - 1
            bigl0 = spool.tile([128, S], f8, tag="stream", name=f"big_0_{cl}")
            for k in range(NSB):
                nc.sync.dma_start(
                    bigl0[:, k * 512 : (k + 1) * 512],
                    srcT8[0, cl * 128 : (cl + 1) * 128, k * 512 : (k + 1) * 512],
                )
                emit_score_mms(0, ps0, cl, bigl0, only_block=k)

            # ---------------- phase 0 tail: p, s0 ----------------------------
            # sigmoid is computed via EXP + DVE math and the gaussian square
            # via a DVE multiply so the ACT engine never swaps its function
            # table mid-kernel (each swap is a ~1.3us ACT_TABLE_LOAD).
            hp_sb = cpool.tile([BEX, H], f32, tag="hp_sb")
            nc.vector.tensor_tensor(hp_sb[:, 0:512], ps_hp0[:], bp_sb[:, 0:512], OP.add)
            nc.vector.tensor_tensor(hp_sb[:, 512:1024], ps_hp1[:], bp_sb[:, 512:1024], OP.add)
            nc.scalar.activation(hp_sb[:], hp_sb[:], AF.Tanh)
            ttr_scr = cpool.tile([BEX, H], f32, tag="ttr_scr")
            pre = cpool.tile([BEX, 1], f32, tag="pre")
            nc.vector.tensor_tensor(ttr_scr[:], hp_sb[:], v_b[:], OP.mult)
            nc.vector.tensor_reduce(pre[:], ttr_scr[:], AX.X, OP.add)
            pv = cpool.tile([BEX, 1], f32, tag="pv")
            nc.scalar.activation(pv[:], pre[:], AF.Exp, bias=nbv_sb, scale=-1.0)
            nc.vector.tensor_scalar(pv[:], pv[:], 1.0, None, OP.add)
            nc.vector.reciprocal(pv[:], pv[:])
            nc.vector.tensor_scalar(pv[:], pv[:], float(S), None, OP.mult)

            s0f = cpool.tile([BEX, 1], f32, tag="s0f")
            nc.vector.tensor_scalar(s0f[:], pv[:], float(WIN // 2), None, OP.subtract)
            nc.vector.tensor_scalar(s0f[:], s0f[:], 0.0, S0MAX, OP.max, OP.min)
            s0i = cpool.tile([BEX, 1], i32, tag="s0i")
            nc.vector.tensor_copy(s0i[:], s0f[:])
            s0ff = cpool.tile([BEX, 1], f32, tag="s0ff")
            nc.vector.tensor_copy(s0ff[:], s0i[:])
            spd = cpool.tile([BEX, 1], f32, tag="spd")
            nc.vector.tensor_tensor(spd[:], s0ff[:], pv[:], OP.subtract)

            # ---------------- ex0 denominator (ACT queue, ungated) -----------
            def stats_phase(e, ps):
                # softmax denominator over full S (constant shift, no max);
                # column j of sums4 holds block 2j sums in rows 0:64 and
                # block 2j+1 sums in rows 64:128. Halves are summed host-side.
                sums4 = mpool.tile([128, NBK], f32, tag="sums4", name=f"sums4_{e}")
                for j in range(NBK):
                    ej = mpool.tile([128, 512], f32, tag="expjunk", name=f"ej_{e}_{j}")
                    nc.scalar.activation(
                        ej[:],
                        ps[j][:],
                        AF.Exp,
                        bias=ebias[:],
                        scale=SCALE,
                        accum_out=sums4[:, j : j + 1],
                    )
                z4 = mpool.tile([128, 1], f32, tag="z4", name=f"z4_{e}", bufs=2)
                nc.vector.tensor_reduce(z4[:], sums4[:], AX.X, OP.add)
                return z4

            # ---------------- p-dependent DMAs (ACT queue, before stats0 so
            # the window pipeline overlaps ex1's streaming) -------------------
            nc.scalar.dma_start(scr_sp[:], spd[:])
            s0_regs = []
            for e in range(BEX):
                s0_regs.append(
                    nc.values_load(
                        s0i[e : e + 1, 0:1],
                        engines=[ET.SP],
                        min_val=0,
                        max_val=int(S0MAX),
                        skip_runtime_bounds_check=True,
                    )
                )

            gauss = []
            for e in range(BEX):
                sp_b = cpool.tile([128, 1], f32, tag=f"sp_b{e}")
                nc.scalar.dma_start(sp_b[:], scr_sp[e : e + 1, 0:1].to_broadcast((128, 1)))
                d = mpool.tile([128, WIN], f32, tag="d", name=f"d_{e}")
                nc.vector.tensor_scalar(d[:], iota_f[:], sp_b[:], None, OP.add)
                nc.vector.tensor_tensor(d[:], d[:], d[:], OP.mult)
                g = cpool.tile([128, WIN], f32, tag=f"gauss{e}")
                nc.scalar.activation(g[:], d[:], AF.Exp, scale=GEXP)
                gauss.append(g)

            # window re-reads ride the SP queue behind the stream dispatches —
            # keeping them off the ACT FIFO lets the denominator exps (which
            # free ex1's score banks) fire the moment ex0's scores land
            wins_all = []
            for e in range(BEX):
                wins = []
                for cc in range(2):
                    winp = winpool.tile(
                        [128, 4, WIN], bf16, tag="win", name=f"win_{e}_{cc}", bufs=4
                    )
                    nc.sync.dma_start(
                        winp[:],
                        srcTb[e, 512 * cc : 512 * (cc + 1), ds(s0_regs[e], WIN)].rearrange(
                            "(c p) w -> p c w", p=128
                        ),
                    )
                    wins.extend([winp[:, i, :] for i in range(4)])
                wins_all.append(wins)

            z4_0 = stats_phase(0, ps0)

            # ---------------- ex1 scores on SP queue (still ungated) ---------
            psw0 = psB.tile([128, WIN], f32, tag="psw", name="win_ps_0", bufs=2)
            psw1 = psB.tile([128, WIN], f32, tag="psw", name="win_ps_1", bufs=2)
            ps1 = [
                psB.tile([128, 512], f32, tag=f"scA{j}", name=f"scA{j}_1")
                for j in range(NBK)
            ]
            for c in range(NH - 1):
                big = spool.tile([128, S], f8, tag="stream", name=f"big_1_{c}")
                nc.sync.dma_start(big[:], srcT8[1, c * 128 : (c + 1) * 128, :])
                emit_score_mms(1, ps1, c, big)
                # window matmuls ride the stream-gap slots late in the loop
                if c == 5:
                    for cc in range(NH):
                        nc.tensor.matmul(
                            psw0[:], t_rep16[0][cc][:], wins_all[0][cc],
                            start=(cc == 0), stop=(cc == NH - 1),
                        )
                elif c == 6:
                    for cc in range(NH):
                        nc.tensor.matmul(
                            psw1[:], t_rep16[1][cc][:], wins_all[1][cc],
                            start=(cc == 0), stop=(cc == NH - 1),
                        )
            # last h-chunk streams in 8 column pieces so each bank's final
            # matmul (and its denominator exp) fires as its piece lands
            cl = NH - 1
            bigl = spool.tile([128, S], f8, tag="stream", name=f"big_1_{cl}")
            for k in range(NSB):
                nc.sync.dma_start(
                    bigl[:, k * 512 : (k + 1) * 512],
                    srcT8[1, cl * 128 : (cl + 1) * 128, k * 512 : (k + 1) * 512],
                )
                emit_score_mms(1, ps1, cl, bigl, only_block=k)

            def build_ctx(e, psw, wins, gauss_e):
                # bf16 window scores -> attention weights -> unnormalized ctx
                expw = mpool.tile([128, WIN], f32, tag="expw", name=f"expw_{e}")
                nc.scalar.activation(expw[:], psw[:], AF.Exp, bias=ebias[:], scale=SCALE)
                attnw = mpool.tile([128, WIN], f32, tag="attnw", name=f"attnw_{e}")
                nc.vector.tensor_tensor(attnw[:], expw[:], gauss_e[:], OP.mult)

                ctx = mpool.tile([128, NH + 1], f32, tag="ctx", name=f"ctx_{e}")
                for c in range(NH):
                    winf = mpool.tile(
                        [128, WIN], f32, tag="winf", name=f"winf_{e}_{c}", bufs=4
                    )
                    nc.vector.tensor_copy(winf[:], wins[c])
                    scr = mpool.tile(
                        [128, WIN], f32, tag="scr512", name=f"scr_{e}_{c}", bufs=4
                    )
                    if c % 2 == 0:
                        nc.vector.tensor_tensor(scr[:], winf[:], attnw[:], OP.mult)
                        ejc = mpool.tile(
                            [128, WIN], f32, tag="ctxjunk", name=f"cj_{e}_{c}", bufs=2
                        )
                        nc.scalar.activation(
                            ejc[:], scr[:], AF.Identity, accum_out=ctx[:, c : c + 1]
                        )
                    else:
                        nc.gpsimd.tensor_tensor(scr[:], winf[:], attnw[:], OP.mult)
                        nc.vector.tensor_reduce(ctx[:, c : c + 1], scr[:], AX.X, OP.add)
                return ctx

            def finish(e, ctx, z4):
                # the denominator rides along as column NH of the output
                nc.vector.tensor_copy(ctx[:, NH : NH + 1], z4[:])
                nc.scalar.dma_start(out[e], ctx[:])

            ctx0 = build_ctx(0, psw0, wins_all[0], gauss[0])
            finish(0, ctx0, z4_0)
            ctx1 = build_ctx(1, psw1, wins_all[1], gauss[1])
            z4_1 = stats_phase(1, ps1)
            finish(1, ctx1, z4_1)

    nc.compile()
    return nc


def _get_nc():
    if "nc" not in _CACHE:
        _CACHE["nc"] = _build()
    return _CACHE["nc"]


def _make_in_maps(src, tgt, wp, bp, vp, bv):
    import ml_dtypes

    srcT = np.ascontiguousarray(src.transpose(0, 2, 1))  # [B, H, S]
    srcT8 = srcT.astype(ml_dtypes.float8_e4m3)
    srcTb = srcT.astype(ml_dtypes.bfloat16)
    in_maps = []
    for k in range(N_CORES):
        lo, hi = k * BEX, (k + 1) * BEX
        tgtTp = np.ascontiguousarray(
            tgt[lo:hi].reshape(BEX, NH, 128).transpose(2, 1, 0)
        )  # [128, NH, BEX] — partition-major so the DMA descriptors are wide
        # pre-replicated stationaries: trep[p, e, c, m] = tgt[e, c*128 + p]
        trep = np.broadcast_to(
            tgt[lo:hi].reshape(BEX, NH, 128).transpose(2, 0, 1)[:, :, :, None],
            (128, BEX, NH, 128),
        )
        in_maps.append(
            {
                "srcT8": srcT8[lo:hi],
                "srcTb": srcTb[lo:hi],
                "trep8": np.ascontiguousarray(trep).astype(ml_dtypes.float8_e4m3),
                "trep16": np.ascontiguousarray(trep).astype(ml_dtypes.bfloat16),
                "tgtT": tgtTp,
                "wp": wp,
                "pvb": np.broadcast_to(
                    np.concatenate([bp[0], vp[0], bv[0], -bv[0]])[None, :],
                    (BEX, 2 * H + 2),
                ).copy(),
            }
        )
    return in_maps


def kernel(source_hidden_states, target_hidden_state, W_p, b_p, v_p, b_v):
    from concourse.bass_utils import run_bass_kernel_spmd

    src = np.asarray(source_hidden_states, dtype=np.float32)
    tgt = np.asarray(target_hidden_state, dtype=np.float32)
    wp = np.asarray(W_p, dtype=np.float32)
    bp = np.asarray(b_p, dtype=np.float32).reshape(1, H)
    vp = np.asarray(v_p, dtype=np.float32).reshape(1, H)
    bv = np.asarray(b_v, dtype=np.float32).reshape(1, 1)

    nc = _get_nc()
    in_maps = _make_in_maps(src, tgt, wp, bp, vp, bv)
    r = run_bass_kernel_spmd(nc, in_maps, list(range(N_CORES)))
    return _unshard(r.results)


def _unshard(results):
    # out[e] is [128, NH+1]: unnormalized ctx in columns 0:NH (context[b, h]
    # with h = c*128 + p lives at out[b, p, c]) and the softmax denominator
    # halves in column NH (row 0 + row 64). Divide through host-side.
    outs = []
    for k in range(N_CORES):
        o = results[k]["out"]
        ctx = o[:, :, 0:NH].transpose(0, 2, 1).reshape(BEX, H)
        Z = o[:, 0, NH : NH + 1] + o[:, 64, NH : NH + 1]
        outs.append(ctx / Z)
    return np.concatenate(outs, axis=0)
